# revision 1
# baseline (speedup 1.0000x reference)
"""Trainium2 Bass kernel for a DAT-style transformer block (sparse_attention).

kernel(**inputs) takes FULL unsharded inputs (B=64), shards the batch across
8 NeuronCores (8 per core, pure data parallel — no collectives), runs one SPMD
Bass/Tile program, returns the FULL [64, 196, 768] float32 output.

Per-core pipeline, software-pipelined over 4 batch-pairs:
  A(p): LN1 + PE transposes -> xnT (spilled to DRAM) + xnTb; q = Wq@xnT
        (padded [16,16] spatial layout); depthwise 3x3 conv via 9 diag
        matmuls; offset head (group-LN -> GELU -> proj -> tanh -> bilinear
        taps); gpsimd gather + weighting -> sampled.
  B(p): k/vT projections; attention in S^T form (S^T = k^T q, exp, AV;
        head-parity softmax sums via selector matmuls; one normalize at
        the end); o-proj + residual + LN2; MLP (streamed W1/W2) + residual.
Emission order A0 A1 B0 A2 B1 A3 B2 B3 overlaps the DVE-heavy A stages
with the PE-heavy B stages. All matmuls bf16 with fp32 PSUM accumulation.
"""

import numpy as np
import ml_dtypes

import concourse.bass as bass
import concourse.mybir as mybir
import concourse.tile as tile
from concourse import library_config
from concourse.bass_utils import run_bass_kernel_spmd

FP32 = mybir.dt.float32
BF16 = mybir.dt.bfloat16
I16 = mybir.dt.int16
AF = mybir.ActivationFunctionType
ALU = mybir.AluOpType

B = 64
NCORES = 8
BL = 8
N = 196
C = 768
NCH = 6
HEADS = 12
HD = 64
G = 8
CG = 96
MLPD = 3072
MMCH = 24
HH = 14
NCK = [(0, 128), (128, 68)]
EPS = 1e-6
OFF_EPS = 1e-5
NPAD = 208
NTAP = 4
QPW = 290


def _f32(x):
    return np.ascontiguousarray(np.asarray(x), dtype=np.float32)


def _bf16(x):
    return np.ascontiguousarray(
        np.asarray(x, dtype=np.float32).astype(ml_dtypes.bfloat16))


def build_host_consts(inp):
    h = {}
    h['WqT'] = _bf16(np.asarray(inp['Wq'], np.float32).T)
    h['WkT'] = _bf16(np.asarray(inp['Wk'], np.float32).T)
    h['WvT'] = _bf16(np.asarray(inp['Wv'], np.float32).T)
    h['WoT'] = _bf16(np.asarray(inp['Wo'], np.float32).T)
    h['W1T'] = _bf16(np.asarray(inp['W1'], np.float32).T)
    h['W2T'] = _bf16(np.asarray(inp['W2'], np.float32).T)

    h['bq'] = _f32(np.asarray(inp['bq']).reshape(NCH, 128).T)
    h['bk'] = _f32(np.asarray(inp['bk']).reshape(NCH, 128).T)
    h['bo'] = _f32(np.asarray(inp['bo']).reshape(NCH, 128).T)
    h['b1'] = _f32(np.asarray(inp['b1']).reshape(MMCH, 128).T)
    h['b2'] = _f32(np.asarray(inp['b2']).reshape(NCH, 128).T)

    dw = np.asarray(inp['off_dw_w'], np.float32).reshape(CG, 9)
    dwg = np.tile(dw, (G, 1))
    diag = np.zeros((9, NCH, 128, 128), np.float32)
    for t in range(9):
        for cc in range(NCH):
            np.fill_diagonal(diag[t, cc], dwg[cc * 128:(cc + 1) * 128, t])
    h['dwdiag'] = _bf16(diag)
    h['dwb'] = _f32(np.tile(np.asarray(inp['off_dw_b'], np.float32), G)
                    .reshape(NCH, 128).T)

    e8 = np.zeros((G, C), np.float32)
    for c in range(C):
        e8[c // CG, c] = 1.0
    h['E8'] = _bf16(e8)

    sel = np.zeros((2, 128, 13 * 128), np.float32)
    for i, (off, nsz) in enumerate(NCK):
        for nl in range(nsz):
            f, p16 = divmod(off + nl, 16)
            for band in range(8):
                sel[i, nl, f * 128 + band * 16 + p16] = 1.0
    h['SelW'] = _bf16(sel)

    sel2 = np.zeros((2, 128), np.float32)
    sel2[0, 0:64] = 1.0
    sel2[1, 64:128] = 1.0
    h['sel2'] = _bf16(sel2)

    # [p, 4] = [1,0,0,1] rows: stationary selectors for head-parity sums
    h['onepm'] = _bf16(np.tile(np.array([1.0, 0.0, 0.0, 1.0], np.float32),
                               (128, 1)))

    ii = np.arange(HH, dtype=np.float32)
    h['refy'] = _f32(np.repeat((ii + 0.5) * 13.0 / 14.0, HH))
    h['refx'] = _f32(np.tile((ii + 0.5) * 13.0 / 14.0, HH))
    h['rowi'] = _f32(np.repeat(ii, HH))
    h['colj'] = _f32(np.tile(ii, HH))

    pw = np.asarray(inp['off_proj_w'], np.float32)
    h['wyv'] = _bf16(np.tile(pw[0], G))
    h['wxv'] = _bf16(np.tile(pw[1], G))

    h['id32'] = _f32(np.eye(128, dtype=np.float32))
    h['id16'] = _bf16(np.eye(128, dtype=np.float32))

    for nm, gk, bk_ in (('ln1', 'ln1_g', 'ln1_b'), ('ln2', 'ln2_g', 'ln2_b')):
        g = np.asarray(inp[gk], np.float32)
        bb = np.asarray(inp[bk_], np.float32)
        h[nm + '_trivial'] = bool(np.all(g == 1.0) and np.all(bb == 0.0))
        h[nm + '_g'] = _f32(g)
        h[nm + '_b'] = _f32(bb)
    og = np.tile(np.asarray(inp['off_ln_g'], np.float32), G)
    ob = np.tile(np.asarray(inp['off_ln_b'], np.float32), G)
    h['offln_trivial'] = bool(np.all(og == 1.0) and np.all(ob == 0.0))
    h['offln_g'] = _f32(og)
    h['offln_b'] = _f32(ob)
    bv = np.asarray(inp['bv'], np.float32)
    h['bv_trivial'] = bool(np.all(bv == 0.0))
    h['bv'] = _f32(np.tile(bv.reshape(1, C), (128, 1)))
    return h


def _free_bcast(t_ap, inner):
    """View [P, F] AP as [P, F, inner] with a stride-0 inner dim."""
    return bass.AP(tensor=t_ap.tensor, offset=t_ap.offset,
                   ap=list(t_ap.ap) + [[0, inner]])


def _dram_bcast(src_ap, rows):
    return bass.AP(tensor=src_ap.tensor, offset=src_ap.offset,
                   ap=[[0, rows]] + list(src_ap.ap))


def emit(nc, tc, d, out_dram, x2_dram, xnT_dram, h):
    x_in = d['x_shard']

    with (
        tc.tile_pool(name='cw', bufs=1) as cw,
        tc.tile_pool(name='pair', bufs=2) as pp,
        tc.tile_pool(name='wstream', bufs=4) as ws,
        tc.tile_pool(name='tmp', bufs=3) as tp,
        tc.tile_pool(name='tbig', bufs=3) as tb,
        tc.tile_pool(name='p_sm', bufs=4) as sm,
        tc.tile_pool(name='ps_mm', bufs=5, space='PSUM') as psm,
        tc.tile_pool(name='ps_tp', bufs=3, space='PSUM') as pst,
    ):
        # ---- resident constants --------------------------------------
        wqkvo = {}
        for wn in ('WqT', 'WkT', 'WvT', 'WoT'):
            wt = cw.tile([128, NCH, C], BF16, tag=wn, name=wn)
            nc.sync.dma_start(
                out=wt[:],
                in_=d[wn][:].rearrange('(k p) c -> p k c', k=NCH))
            wqkvo[wn] = wt
        WqT = [wqkvo['WqT'][:, k, :] for k in range(NCH)]
        WkT = [wqkvo['WkT'][:, k, :] for k in range(NCH)]
        WvT = [wqkvo['WvT'][:, k, :] for k in range(NCH)]
        WoT = [wqkvo['WoT'][:, k, :] for k in range(NCH)]
        E8 = cw.tile([G, C], BF16, tag='e8', name='e8')
        nc.sync.dma_start(out=E8[:], in_=d['E8'][:])
        sel2_t = cw.tile([2, 128], BF16, tag='sel2', name='sel2')
        nc.sync.dma_start(out=sel2_t[:], in_=d['sel2'][:])
        onepm_t = cw.tile([128, 4], BF16, tag='onepm', name='onepm')
        nc.sync.dma_start(out=onepm_t[:], in_=d['onepm'][:])
        SelW = [cw.tile([128, 13 * 128], BF16, tag=f'sel{i}', name=f'sel{i}')
                for i in range(2)]
        for i in range(2):
            nc.sync.dma_start(out=SelW[i][:], in_=d['SelW'][i])
        id32 = cw.tile([128, 128], FP32, tag='id32', name='id32')
        id16 = cw.tile([128, 128], BF16, tag='id16', name='id16')
        nc.sync.dma_start(out=id32[:], in_=d['id32'][:])
        nc.sync.dma_start(out=id16[:], in_=d['id16'][:])
        bias_t = {}
        for nm, cols in (('bq', NCH), ('bk', NCH), ('bo', NCH), ('b1', MMCH),
                         ('b2', NCH), ('dwb', NCH)):
            bias_t[nm] = cw.tile([128, cols], FP32, tag='bias_' + nm,
                                 name='bias_' + nm)
            nc.sync.dma_start(out=bias_t[nm][:], in_=d[nm][:])
        refy_t, refx_t, rowi_t, colj_t = [], [], [], []
        for i, (off, nsz) in enumerate(NCK):
            for nm, lst in (('refy', refy_t), ('refx', refx_t),
                            ('rowi', rowi_t), ('colj', colj_t)):
                tt = cw.tile([nsz, 1], FP32, tag=f'{nm}{i}', name=f'{nm}{i}')
                nc.sync.dma_start(
                    out=tt[:],
                    in_=d[nm][off:off + nsz].rearrange('(n one) -> n one',
                                                       one=1))
                lst.append(tt)
        wyb = cw.tile([128, C], BF16, tag='wyb', name='wyb')
        wxb = cw.tile([128, C], BF16, tag='wxb', name='wxb')
        nc.sync.dma_start(out=wyb[:], in_=_dram_bcast(d['wyv'][:], 128))
        nc.sync.dma_start(out=wxb[:], in_=_dram_bcast(d['wxv'][:], 128))
        eps_t = cw.tile([128, 1], FP32, tag='eps', name='eps')
        nc.vector.memset(eps_t[:], EPS)
        oeps_t = cw.tile([128, 1], FP32, tag='oeps', name='oeps')
        nc.vector.memset(oeps_t[:], OFF_EPS)
        gbt = {}
        for nm in ('ln1', 'ln2', 'offln'):
            if not h[nm + '_trivial']:
                g_ = cw.tile([128, C], FP32, tag=nm + 'g', name=nm + 'g')
                b_ = cw.tile([128, C], FP32, tag=nm + 'b', name=nm + 'b')
                nc.sync.dma_start(out=g_[:], in_=_dram_bcast(d[nm + '_g'][:], 128))
                nc.sync.dma_start(out=b_[:], in_=_dram_bcast(d[nm + '_b'][:], 128))
                gbt[nm] = (g_, b_)
        bv_t = None
        if not h['bv_trivial']:
            bv_t = cw.tile([128, C], FP32, tag='bvt', name='bvt')
            nc.sync.dma_start(out=bv_t[:], in_=d['bv'][:])

        def ln_norm(xf, nsz, out_ap, gbk):
            st = sm.tile([128, 3, 6], FP32, tag='ln_st', name='ln_st')
            for s in range(3):
                nc.vector.bn_stats(out=st[:nsz, s, :],
                                   in_=xf[:nsz, s * 256:(s + 1) * 256])
            mv = sm.tile([128, 2], FP32, tag='ln_mv', name='ln_mv')
            nc.vector.bn_aggr(out=mv[:nsz], in_=st[:nsz])
            std = sm.tile([128, 1], FP32, tag='ln_std', name='ln_std')
            nc.scalar.activation(out=std[:nsz], in_=mv[:nsz, 1:2], func=AF.Sqrt,
                                 bias=eps_t[:nsz], scale=1.0)
            rstd = sm.tile([128, 1], FP32, tag='ln_rstd', name='ln_rstd')
            nc.vector.reciprocal(out=rstd[:nsz], in_=std[:nsz])
            nmr = sm.tile([128, 1], FP32, tag='ln_nmr', name='ln_nmr')
            nc.vector.tensor_scalar(out=nmr[:nsz], in0=mv[:nsz, 0:1],
                                    scalar1=rstd[:nsz], scalar2=-1.0,
                                    op0=ALU.mult, op1=ALU.mult)
            if gbk is None:
                nc.scalar.activation(out=out_ap, in_=xf[:nsz], func=AF.Identity,
                                     bias=nmr[:nsz], scale=rstd[:nsz])
            else:
                gt, bt = gbk
                tmp = tb.tile([128, C], FP32, tag='xb', name='ln_tmp')
                nc.scalar.activation(out=tmp[:nsz], in_=xf[:nsz], func=AF.Identity,
                                     bias=nmr[:nsz], scale=rstd[:nsz])
                nc.vector.tensor_mul(out=tmp[:nsz], in0=tmp[:nsz], in1=gt[:nsz])
                nc.vector.tensor_add(out=out_ap, in0=tmp[:nsz], in1=bt[:nsz])

        # per-pair state handed from A(p) to B(p)
        state = {}

        def q_mov(qp_p, hp, bi, p0, np_=64):
            """[np_, 14, 14] moving-operand view of the padded q layout."""
            base = qp_p[hp][p0:p0 + np_, bi * QPW + 17:bi * QPW + 18]
            return bass.AP(tensor=base.tensor, offset=base.offset,
                           ap=[base.ap[0], [16, 14], [1, 14]])

        def stA(p):
            bp = 2 * p
            # ---------------- ST1: LN1 + transposes -------------------
            xnTb = [pp.tile([128, 2, N], BF16, tag=f'xnTb{k}',
                            name=f'xnTb{k}', bufs=1) for k in range(NCH)]
            for bi, b in enumerate((bp, bp + 1)):
                xnt_tmp = tb.tile([128, NCH, N], FP32, tag='img',
                                  name='st1_xnT', bufs=3)
                for i, (off, nsz) in enumerate(NCK):
                    xf = tb.tile([128, C], FP32, tag='xa', name='st1_x', bufs=2)
                    nc.sync.dma_start(out=xf[:nsz], in_=x_in[b, off:off + nsz, :])
                    xn = tb.tile([128, C], FP32, tag='xb', name='st1_xn')
                    ln_norm(xf, nsz, xn[:nsz], gbt.get('ln1'))
                    for cc in range(NCH):
                        pt = pst.tile([128, 128], FP32, tag='tp',
                                      name='st1_ps')
                        nc.tensor.transpose(
                            pt[:, :nsz], xn[:nsz, cc * 128:(cc + 1) * 128],
                            id32[:nsz, :nsz])
                        nc.scalar.activation(
                            out=xnt_tmp[:, cc, off:off + nsz],
                            in_=pt[:, :nsz], func=AF.Identity)
                        nc.vector.tensor_copy(
                            out=xnTb[cc][:, bi, off:off + nsz],
                            in_=pt[:, :nsz])
                nc.scalar.dma_start(out=xnT_dram[b], in_=xnt_tmp[:])

            # ---------------- ST2: q ----------------------------------
            qp_p = [pp.tile([128, 2 * QPW], BF16, tag=f'qp{k}',
                            name=f'qp{k}', bufs=2) for k in range(NCH)]
            for k in range(NCH):
                nc.vector.memset(qp_p[k][:], 0.0)
            for oc in range(NCH):
                q_ps = psm.tile([128, 392], FP32, tag='mm', name='st2_ps')
                for kc in range(NCH):
                    nc.tensor.matmul(
                        q_ps[:], WqT[kc][:, oc * 128:(oc + 1) * 128],
                        xnTb[kc][:], start=(kc == 0), stop=(kc == NCH - 1))
                for bi in range(2):
                    base = qp_p[oc][:, bi * QPW + 17:bi * QPW + 18]
                    outap = bass.AP(tensor=base.tensor, offset=base.offset,
                                    ap=[base.ap[0], [16, 14], [1, 14]])
                    nc.scalar.activation(
                        out=outap, in_=q_ps[:, bi * N:(bi + 1) * N],
                        func=AF.Identity, bias=bias_t['bq'][:, oc:oc + 1])

            # ---------------- ST3: depthwise conv ---------------------
            dwc = [ws.tile([128, 9, 128], BF16, tag='dwc', name=f'dwc{oc}',
                           bufs=2) for oc in range(NCH)]
            dd = d['dwdiag']
            for oc in range(NCH):
                base = dd[0, oc]
                nc.sync.dma_start(
                    out=dwc[oc][:],
                    in_=bass.AP(tensor=base.tensor, offset=base.offset,
                                ap=[[128, 128], [NCH * 128 * 128, 9],
                                    [1, 128]]))
            ocT = {}
            for bi, b in enumerate((bp, bp + 1)):
                ocT[b] = tp.tile([128, 2, C], BF16, tag='st3_ocT',
                                 name='st3_ocT', bufs=2)
            for oc in range(NCH):
                for bi, b in enumerate((bp, bp + 1)):
                    cv_ps = psm.tile([128, 256], FP32, tag='mm',
                                     name='st3_ps')
                    for t in range(9):
                        ky, kx = divmod(t, 3)
                        d0 = bi * QPW + 16 * ky + kx
                        nc.tensor.matmul(
                            cv_ps[:], dwc[oc][:, t, :],
                            qp_p[oc][:, d0:d0 + 256],
                            start=(t == 0), stop=(t == 8))
                    cvb = tp.tile([128, N], BF16, tag='st3_cvb',
                                  name='st3_cvb', bufs=2)
                    base = cv_ps[:, 0:1]
                    inap = bass.AP(tensor=base.tensor, offset=base.offset,
                                   ap=[base.ap[0], [16, 14], [1, 14]])
                    nc.scalar.activation(out=cvb[:], in_=inap,
                                         func=AF.Identity,
                                         bias=bias_t['dwb'][:, oc:oc + 1])
                    for i, (off, nsz) in enumerate(NCK):
                        pt = pst.tile([128, 128], BF16, tag='tp',
                                      name='st3_tp')
                        nc.tensor.transpose(pt[:nsz, :], cvb[:, off:off + nsz],
                                            id16)
                        nc.vector.tensor_copy(
                            out=ocT[b][:nsz, i, oc * 128:(oc + 1) * 128],
                            in_=pt[:nsz, :])

            # ---------------- ST4: offset head ------------------------
            idx4p = [sm.tile([128, 2, NTAP, G], BF16, tag=f'idx4_{i}',
                             name=f'idx4_{i}', bufs=2) for i in range(2)]
            W48 = {}
            for bi, b in enumerate((bp, bp + 1)):
                W48[b] = sm.tile([G, NTAP * NPAD], BF16, tag='w48',
                                 name='w48', bufs=2)
                nc.vector.memset(W48[b][:], 0.0)
                for i, (off, nsz) in enumerate(NCK):
                    sl = ocT[b][:nsz, i, :]
                    st8 = sm.tile([128, G, 6], FP32, tag='off_st',
                                  name='off_st')
                    mv8 = sm.tile([128, G, 2], FP32, tag='off_mv',
                                  name='off_mv')
                    for g in range(G):
                        nc.vector.bn_stats(out=st8[:nsz, g, :],
                                           in_=sl[:, g * CG:(g + 1) * CG])
                        nc.vector.bn_aggr(out=mv8[:nsz, g, :],
                                          in_=st8[:nsz, g, :])
                    std8 = sm.tile([128, G], FP32, tag='off_std',
                                   name='off_std')
                    nc.scalar.activation(out=std8[:nsz], in_=mv8[:nsz, :, 1],
                                         func=AF.Sqrt, bias=oeps_t[:nsz],
                                         scale=1.0)
                    rec8 = sm.tile([128, G], FP32, tag='off_rec',
                                   name='off_rec')
                    nc.vector.reciprocal(out=rec8[:nsz], in_=std8[:nsz])
                    og = tp.tile([128, C], BF16, tag='off_og', name='off_og',
                                 bufs=2)
                    ogv = og[:nsz].rearrange('p (g c) -> p g c', g=G)
                    nc.vector.tensor_tensor(
                        out=ogv, in0=sl.rearrange('p (g c) -> p g c', g=G),
                        in1=_free_bcast(mv8[:nsz, :, 0], CG),
                        op=ALU.subtract)
                    nc.vector.tensor_tensor(out=ogv, in0=ogv,
                                            in1=_free_bcast(rec8[:nsz], CG),
                                            op=ALU.mult)
                    if not h['offln_trivial']:
                        gt, bt = gbt['offln']
                        nc.vector.tensor_mul(out=og[:nsz], in0=og[:nsz],
                                             in1=gt[:nsz])
                        nc.vector.tensor_add(out=og[:nsz], in0=og[:nsz],
                                             in1=bt[:nsz])
                    nc.scalar.activation(out=og[:nsz], in_=og[:nsz],
                                         func=AF.Gelu)
                    oyx = sm.tile([128, 16], FP32, tag='off_oyx',
                                  name='off_oyx')
                    tpm = tp.tile([128, C], BF16, tag='off_og',
                                  name='off_tpm', bufs=2)
                    nc.vector.tensor_mul(out=tpm[:nsz], in0=og[:nsz],
                                         in1=wyb[:nsz])
                    nc.vector.tensor_reduce(
                        out=oyx[:nsz, 0:G],
                        in_=tpm[:nsz].rearrange('p (g c) -> p g c', g=G),
                        axis=mybir.AxisListType.X, op=ALU.add)
                    nc.vector.tensor_mul(out=tpm[:nsz], in0=og[:nsz],
                                         in1=wxb[:nsz])
                    nc.vector.tensor_reduce(
                        out=oyx[:nsz, G:16],
                        in_=tpm[:nsz].rearrange('p (g c) -> p g c', g=G),
                        axis=mybir.AxisListType.X, op=ALU.add)
                    th = sm.tile([128, 16], FP32, tag='off_th', name='off_th')
                    nc.scalar.activation(out=th[:nsz], in_=oyx[:nsz],
                                         func=AF.Tanh)
                    gy = sm.tile([128, G], FP32, tag='off_gy', name='off_gy')
                    gx = sm.tile([128, G], FP32, tag='off_gx', name='off_gx')
                    nc.vector.tensor_scalar(out=gy[:nsz], in0=th[:nsz, 0:G],
                                            scalar1=6.5 / 14.0,
                                            scalar2=refy_t[i][:],
                                            op0=ALU.mult, op1=ALU.add)
                    nc.vector.tensor_scalar(out=gx[:nsz], in0=th[:nsz, G:16],
                                            scalar1=6.5 / 14.0,
                                            scalar2=refx_t[i][:],
                                            op0=ALU.mult, op1=ALU.add)
                    fy = sm.tile([128, G], FP32, tag='off_fy', name='off_fy')
                    fx = sm.tile([128, G], FP32, tag='off_fx', name='off_fx')
                    y0 = sm.tile([128, G], FP32, tag='off_y0', name='off_y0')
                    x0 = sm.tile([128, G], FP32, tag='off_x0', name='off_x0')
                    # floor(gy) = rowi - [gy < rowi]  (exact: |offset|<0.5px)
                    nc.vector.tensor_scalar(out=y0[:nsz], in0=gy[:nsz],
                                            scalar1=rowi_t[i][:], scalar2=None,
                                            op0=ALU.is_lt)
                    nc.vector.tensor_scalar(out=y0[:nsz], in0=y0[:nsz],
                                            scalar1=-1.0, scalar2=rowi_t[i][:],
                                            op0=ALU.mult, op1=ALU.add)
                    nc.vector.tensor_scalar(out=x0[:nsz], in0=gx[:nsz],
                                            scalar1=colj_t[i][:], scalar2=None,
                                            op0=ALU.is_lt)
                    nc.vector.tensor_scalar(out=x0[:nsz], in0=x0[:nsz],
                                            scalar1=-1.0, scalar2=colj_t[i][:],
                                            op0=ALU.mult, op1=ALU.add)
                    nc.vector.tensor_scalar_min(out=y0[:nsz], in0=y0[:nsz],
                                                scalar1=12.0)
                    nc.vector.tensor_scalar_min(out=x0[:nsz], in0=x0[:nsz],
                                                scalar1=12.0)
                    nc.vector.tensor_sub(out=fy[:nsz], in0=gy[:nsz],
                                         in1=y0[:nsz])
                    nc.vector.tensor_sub(out=fx[:nsz], in0=gx[:nsz],
                                         in1=x0[:nsz])
                    ia = sm.tile([128, G], FP32, tag='off_ia', name='off_ia')
                    nc.vector.scalar_tensor_tensor(out=ia[:nsz], in0=y0[:nsz],
                                                   scalar=14.0, in1=x0[:nsz],
                                                   op0=ALU.mult, op1=ALU.add)
                    nc.vector.tensor_copy(out=idx4p[i][:nsz, bi, 0, :],
                                          in_=ia[:nsz])
                    for t, ofs in ((1, 14.0), (2, 1.0), (3, 15.0)):
                        nc.vector.tensor_scalar_add(
                            out=idx4p[i][:nsz, bi, t, :], in0=ia[:nsz],
                            scalar1=ofs)
                    fy1 = sm.tile([128, G], FP32, tag='off_fy1',
                                  name='off_fy1')
                    fx1 = sm.tile([128, G], FP32, tag='off_fx1',
                                  name='off_fx1')
                    nc.vector.tensor_scalar(out=fy1[:nsz], in0=fy[:nsz],
                                            scalar1=-1.0, scalar2=1.0,
                                            op0=ALU.mult, op1=ALU.add)
                    nc.vector.tensor_scalar(out=fx1[:nsz], in0=fx[:nsz],
                                            scalar1=-1.0, scalar2=1.0,
                                            op0=ALU.mult, op1=ALU.add)
                    for t, (aa, bb2) in enumerate(((fx1, fy1), (fx1, fy),
                                                   (fx, fy1), (fx, fy))):
                        wt = sm.tile([128, G], BF16, tag='off_wt',
                                     name='off_wt')
                        nc.vector.tensor_mul(out=wt[:nsz], in0=aa[:nsz],
                                             in1=bb2[:nsz])
                        ptw = pst.tile([G, 128], BF16, tag='tp',
                                       name='off_ptw')
                        nc.tensor.transpose(ptw[:, :nsz], wt[:nsz],
                                            id16[:nsz, :nsz])
                        nc.vector.tensor_copy(
                            out=W48[b][:, t * NPAD + off:t * NPAD + off + nsz],
                            in_=ptw[:, :nsz])

            # wrap indices into gather layout (split across two PSUM tiles)
            wrapP = [psm.tile([128, 7 * 64], FP32, tag='mm', name='st4_wrap')
                     for _ in range(2)]
            for f in range(13):
                w_t, fo = (wrapP[0], f) if f < 7 else (wrapP[1], f - 7)
                for i in range(2):
                    nsz = NCK[i][1]
                    nc.tensor.matmul(
                        w_t[:, fo * 64:(fo + 1) * 64],
                        SelW[i][:nsz, f * 128:(f + 1) * 128],
                        idx4p[i][:nsz].rearrange('p b t g -> p (b t g)'),
                        start=(i == 0), stop=(i == 1))
            wrapS = sm.tile([128, G, 2, NTAP, 13], I16, tag='wrapS',
                            name='wrapS', bufs=2)
            for bb in range(2):
                for half, (f0, fn) in enumerate(((0, 7), (7, 6))):
                    base_in = wrapP[half][:, 0:1]
                    inap = bass.AP(tensor=base_in.tensor,
                                   offset=base_in.offset + bb * 32,
                                   ap=[base_in.ap[0], [64, fn], [8, NTAP],
                                       [1, G]])
                    base_out = wrapS[:, 0, bb, 0, f0:f0 + 1]
                    outap = bass.AP(tensor=base_out.tensor,
                                    offset=base_out.offset,
                                    ap=[base_out.ap[0], [1, fn], [13, NTAP],
                                        [104, G]])
                    nc.vector.tensor_copy(out=outap, in_=inap)
            idxt = [sm.tile([128, 2, NTAP, 13], I16, tag=f'idxt{j}',
                            name=f'idxt{j}', bufs=2) for j in range(NCH)]
            for j in range(NCH):
                bands_g = [(8 * j + band) // 6 for band in range(8)]
                runs = []
                r0 = 0
                for band in range(1, 9):
                    if band == 8 or bands_g[band] != bands_g[r0]:
                        runs.append((r0, band - 1, bands_g[r0]))
                        r0 = band
                for (b0, b1, g) in runs:
                    p0, pn = 16 * b0, 16 * (b1 - b0 + 1)
                    nc.gpsimd.dma_start(out=idxt[j][p0:p0 + pn],
                                         in_=wrapS[p0:p0 + pn, g])

            # ---------------- ST5: gather + bilinear weighting --------
            sampled = [pp.tile([128, 2, N], BF16, tag=f'smp{k}',
                               name=f'smp{k}', bufs=2) for k in range(NCH)]
            for bi, b in enumerate((bp, bp + 1)):
                img = tb.tile([128, NCH, N], FP32, tag='img', name='st5_img',
                              bufs=3)
                nc.gpsimd.dma_start(out=img[:], in_=xnT_dram[b])
                for j in range(NCH):
                    gws = []
                    for half in range(2):
                        gth = tp.tile([128, 2 * NPAD], FP32, tag='st5_g',
                                      name='st5_g', bufs=3)
                        nc.gpsimd.ap_gather(
                            out_ap=gth[:],
                            in_ap=img[:, j, :].rearrange(
                                'p (n one) -> p n one', one=1),
                            idxs_ap=idxt[j][:, bi, 2 * half:2 * half + 2, :]
                            .rearrange('p t f -> p (t f)'),
                            channels=128, num_elems=N, d=1,
                            num_idxs=2 * NPAD)
                        wb = psm.tile([128, 2 * NPAD], FP32, tag='mm',
                                      name=f'st5_w{half}')
                        nc.tensor.matmul(
                            wb[:], E8[:, j * 128:(j + 1) * 128],
                            W48[b][:, half * 2 * NPAD:(half + 1) * 2 * NPAD],
                            start=True, stop=True)
                        gw = tp.tile([128, 2 * NPAD], BF16, tag='st5_gw',
                                     name='st5_gw', bufs=3)
                        nc.vector.tensor_tensor(out=gw[:], in0=gth[:],
                                                in1=wb[:], op=ALU.mult)
                        gws.append(gw)
                    s01 = tp.tile([128, N], BF16, tag='st5_s01',
                                  name='st5_s01', bufs=2)
                    nc.vector.tensor_add(out=s01[:], in0=gws[0][:, 0:N],
                                         in1=gws[0][:, NPAD:NPAD + N])
                    s23 = tp.tile([128, N], BF16, tag='st5_s23',
                                  name='st5_s23', bufs=2)
                    nc.vector.tensor_add(out=s23[:], in0=gws[1][:, 0:N],
                                         in1=gws[1][:, NPAD:NPAD + N])
                    nc.vector.tensor_add(out=sampled[j][:, bi, :],
                                         in0=s01[:], in1=s23[:])
            state[p] = (qp_p, sampled)

        def stB(p):
            bp = 2 * p
            qp_p, sampled = state.pop(p)
            # ---------------- ST6: k and vT ---------------------------
            k_p = [pp.tile([128, 2, N], BF16, tag=f'kk{k}', name=f'kk{k}',
                           bufs=1) for k in range(NCH)]
            vT_p = pp.tile([128, 2, 2, C], BF16, tag='vT', name='vT', bufs=1)
            for oc in range(NCH):
                k_ps = psm.tile([128, 392], FP32, tag='mm', name='st6_kps')
                for kc in range(NCH):
                    nc.tensor.matmul(
                        k_ps[:], WkT[kc][:, oc * 128:(oc + 1) * 128],
                        sampled[kc][:], start=(kc == 0), stop=(kc == NCH - 1))
                nc.scalar.activation(
                    out=k_p[oc][:], in_=k_ps[:].rearrange('p (b n) -> p b n',
                                                          b=2),
                    func=AF.Identity, bias=bias_t['bk'][:, oc:oc + 1])
            for bi in range(2):
                for i, (off, nsz) in enumerate(NCK):
                    for half in range(2):
                        v_ps = psm.tile([128, 384], FP32, tag='mm',
                                        name='st6_vps')
                        for kc in range(NCH):
                            nc.tensor.matmul(
                                v_ps[:nsz], sampled[kc][:, bi, off:off + nsz],
                                WvT[kc][:, half * 384:(half + 1) * 384],
                                start=(kc == 0), stop=(kc == NCH - 1))
                        dst = vT_p[:nsz, bi, i, half * 384:(half + 1) * 384]
                        if bv_t is None:
                            nc.vector.tensor_copy(out=dst, in_=v_ps[:nsz])
                        else:
                            nc.vector.tensor_add(
                                out=dst, in0=v_ps[:nsz],
                                in1=bv_t[:nsz, half * 384:(half + 1) * 384])

            # ---------------- ST7: attention (S^T form) ---------------
            aop = pp.tile([128, NCH, 2, N], BF16, tag='st7_ao',
                          name='st7_ao', bufs=1)
            for bi, b in enumerate((bp, bp + 1)):
                for hp in range(NCH):
                    o_ps = psm.tile([128, N], FP32, tag='mm', name='st7_ops')
                    sums_ps = pst.tile([2, N], FP32, tag='tp',
                                       name='st7_sums')
                    for hh in range(2):
                        hd = hp * 2 + hh
                        p0 = hh * 64
                        expT = tp.tile([128, 2, N], BF16, tag='st7_exp',
                                       name='st7_exp', bufs=3)
                        s_ps = psm.tile([128, 2, N], FP32, tag='mm',
                                        name='st7_sps')
                        for ni, (noff, nsz) in enumerate(NCK):
                            nc.tensor.matmul(
                                s_ps[:nsz, ni, :],
                                k_p[hp][p0:p0 + 64, bi, noff:noff + nsz],
                                q_mov(qp_p, hp, bi, p0),
                                start=True, stop=True)
                        nc.scalar.activation(out=expT[:], in_=s_ps[:],
                                             func=AF.Exp, scale=0.125)
                        for ni, (noff, nsz) in enumerate(NCK):
                            nc.tensor.matmul(
                                o_ps[p0:p0 + 64, :],
                                vT_p[:nsz, bi, ni, hd * 64:(hd + 1) * 64],
                                expT[:nsz, ni, :],
                                start=(ni == 0), stop=(ni == 1))
                            nc.tensor.matmul(
                                sums_ps[:],
                                onepm_t[:nsz, 2 * hh:2 * hh + 2],
                                expT[:nsz, ni, :],
                                start=(hh == 0 and ni == 0),
                                stop=(hh == 1 and ni == 1))
                    rec2 = sm.tile([2, N], BF16, tag='st7_rec',
                                   name='st7_rec', bufs=4)
                    with nc.allow_low_precision(
                            reason='softmax recip to bf16 for PE bcast; '
                                   '2e-2 tol'):
                        nc.vector.reciprocal(out=rec2[:], in_=sums_ps[:])
                    bc_ps = psm.tile([128, N], FP32, tag='mm', name='st7_bc')
                    nc.tensor.matmul(bc_ps[:], sel2_t[:], rec2[:],
                                     start=True, stop=True)
                    bc_sb = tp.tile([128, N], BF16, tag='st7_bcs',
                                    name='st7_bcs', bufs=2)
                    nc.scalar.activation(out=bc_sb[:], in_=bc_ps[:],
                                         func=AF.Identity)
                    nc.vector.tensor_tensor(out=aop[:, hp, bi, :],
                                            in0=o_ps[:], in1=bc_sb[:],
                                            op=ALU.mult)

            # o-proj, residual 1, LN2, transposes
            xn2T = [pp.tile([128, 2, N], BF16, tag=f'x2T{k}', name=f'x2T{k}',
                            bufs=1) for k in range(NCH)]
            ybf = pp.tile([128, NCH, 2, N], BF16, tag='ybm2',
                          name='st7_ybf', bufs=1)
            for oc in range(NCH):
                y_ps = psm.tile([128, 392], FP32, tag='mm', name='st7_yps')
                for kc in range(NCH):
                    nc.tensor.matmul(
                        y_ps[:], WoT[kc][:, oc * 128:(oc + 1) * 128],
                        aop[:, kc, :, :], start=(kc == 0),
                        stop=(kc == NCH - 1))
                nc.scalar.activation(
                    out=ybf[:, oc, :, :],
                    in_=y_ps[:].rearrange('p (b n) -> p b n', b=2),
                    func=AF.Identity, bias=bias_t['bo'][:, oc:oc + 1])
            for bi, b in enumerate((bp, bp + 1)):
                for i, (off, nsz) in enumerate(NCK):
                    xo = tb.tile([128, C], FP32, tag='xa', name='st7_xo', bufs=2)
                    nc.sync.dma_start(out=xo[:nsz], in_=x_in[b, off:off + nsz, :])
                    x2 = tb.tile([128, C], FP32, tag='xb', name='st7_x2')
                    for oc in range(NCH):
                        ypt = pst.tile([128, 128], BF16, tag='tp',
                                       name='st7_ypt')
                        nc.tensor.transpose(ypt[:nsz, :],
                                            ybf[:, oc, bi, off:off + nsz],
                                            id16)
                        nc.vector.tensor_add(
                            out=x2[:nsz, oc * 128:(oc + 1) * 128],
                            in0=ypt[:nsz, :],
                            in1=xo[:nsz, oc * 128:(oc + 1) * 128])
                    nc.scalar.dma_start(out=x2_dram[b, off:off + nsz, :],
                                        in_=x2[:nsz])
                    xn2 = tb.tile([128, C], FP32, tag='xb', name='st7_xn2')
                    ln_norm(x2, nsz, xn2[:nsz], gbt.get('ln2'))
                    for cc in range(NCH):
                        pt = pst.tile([128, 128], FP32, tag='tp',
                                      name='st7_tps')
                        nc.tensor.transpose(
                            pt[:, :nsz], xn2[:nsz, cc * 128:(cc + 1) * 128],
                            id32[:nsz, :nsz])
                        nc.vector.tensor_copy(
                            out=xn2T[cc][:, bi, off:off + nsz],
                            in_=pt[:, :nsz])

            # ---------------- ST9: MLP + residual 2 -------------------
            h1_all = tp.tile([128, MMCH, 392], BF16, tag='h1_all',
                             name='h1_all', bufs=1)
            w1c = []
            for mm in range(MMCH):
                w1t = ws.tile([128, NCH, 128], BF16, tag='w1c',
                              name=f'w1c{mm}', bufs=2)
                base = d['W1T'][:]
                nc.sync.dma_start(
                    out=w1t[:],
                    in_=bass.AP(tensor=base.tensor,
                                offset=base.offset + mm * 128,
                                ap=[[MLPD, 128], [128 * MLPD, NCH],
                                    [1, 128]]))
                w1c.append(w1t)
                h1_ps = psm.tile([128, 392], FP32, tag='mm', name='h1ps')
                for kc in range(NCH):
                    nc.tensor.matmul(
                        h1_ps[:], w1t[:, kc, :], xn2T[kc][:],
                        start=(kc == 0), stop=(kc == NCH - 1))
                nc.scalar.activation(out=h1_all[:, mm, :], in_=h1_ps[:],
                                     func=AF.Gelu,
                                     bias=bias_t['b1'][:, mm:mm + 1],
                                     scale=1.0)
            m2b = pp.tile([128, NCH, 2, N], BF16, tag='ybm2',
                          name='st9_m2b', bufs=1)
            w2base = d['W2T'][:]
            for oc in range(NCH):
                w2col = ws.tile([128, MMCH, 128], BF16, tag='w2c',
                                name=f'w2col{oc}', bufs=2)
                nc.sync.dma_start(
                    out=w2col[:],
                    in_=bass.AP(tensor=w2base.tensor,
                                offset=w2base.offset + oc * 128,
                                ap=[[C, 128], [128 * C, MMCH], [1, 128]]))
                m2_ps = psm.tile([128, 392], FP32, tag='mm', name='m2ps')
                for mm in range(MMCH):
                    nc.tensor.matmul(
                        m2_ps[:], w2col[:, mm, :], h1_all[:, mm, :],
                        start=(mm == 0), stop=(mm == MMCH - 1))
                nc.scalar.activation(
                    out=m2b[:, oc, :, :],
                    in_=m2_ps[:].rearrange('p (b n) -> p b n', b=2),
                    func=AF.Identity, bias=bias_t['b2'][:, oc:oc + 1])
            for bi, b in enumerate((bp, bp + 1)):
                for i, (off, nsz) in enumerate(NCK):
                    x2r = tb.tile([128, C], FP32, tag='xa', name='st9_x2r', bufs=2)
                    nc.sync.dma_start(out=x2r[:nsz],
                                      in_=x2_dram[b, off:off + nsz, :])
                    ot = tb.tile([128, C], FP32, tag='xb', name='st9_out')
                    for oc in range(NCH):
                        mpt = pst.tile([128, 128], BF16, tag='tp',
                                       name='st9_mpt')
                        nc.tensor.transpose(mpt[:nsz, :],
                                            m2b[:, oc, bi, off:off + nsz],
                                            id16)
                        nc.vector.tensor_add(
                            out=ot[:nsz, oc * 128:(oc + 1) * 128],
                            in0=mpt[:nsz, :],
                            in1=x2r[:nsz, oc * 128:(oc + 1) * 128])
                    nc.scalar.dma_start(out=out_dram[b, off:off + nsz, :],
                                        in_=ot[:nsz])

        # software-pipelined schedule
        stA(0)
        stA(1)
        stB(0)
        stA(2)
        stB(1)
        stA(3)
        stB(2)
        stB(3)


def build_nc(h):
    from concourse import bacc
    nc = bacc.Bacc(None, target_bir_lowering=False, debug=False)
    d = {}

    def din(name, shape, dt):
        d[name] = nc.declare_dram_parameter(name, list(shape), dt, isOutput=False)

    din('x_shard', (BL, N, C), FP32)
    out_dram = nc.declare_dram_parameter('out', [BL, N, C], FP32, isOutput=True)
    x2_dram = nc.dram_tensor('x2_scratch', [BL, N, C], FP32)
    xnT_dram = nc.dram_tensor('xnT_scratch', [BL, 128, NCH, N], FP32)

    din('WqT', (C, C), BF16); din('WkT', (C, C), BF16)
    din('WvT', (C, C), BF16); din('WoT', (C, C), BF16)
    din('W1T', (C, MLPD), BF16); din('W2T', (MLPD, C), BF16)
    din('bq', (128, NCH), FP32); din('bk', (128, NCH), FP32)
    din('bo', (128, NCH), FP32); din('b1', (128, MMCH), FP32)
    din('b2', (128, NCH), FP32)
    din('dwdiag', (9, NCH, 128, 128), BF16); din('dwb', (128, NCH), FP32)
    din('E8', (G, C), BF16); din('SelW', (2, 128, 13 * 128), BF16)
    din('sel2', (2, 128), BF16)
    din('onepm', (128, 4), BF16)
    din('refy', (N,), FP32); din('refx', (N,), FP32)
    din('rowi', (N,), FP32); din('colj', (N,), FP32)
    din('wyv', (C,), BF16); din('wxv', (C,), BF16)
    din('id32', (128, 128), FP32); din('id16', (128, 128), BF16)
    if not h['ln1_trivial']:
        din('ln1_g', (C,), FP32); din('ln1_b', (C,), FP32)
    if not h['ln2_trivial']:
        din('ln2_g', (C,), FP32); din('ln2_b', (C,), FP32)
    if not h['offln_trivial']:
        din('offln_g', (C,), FP32); din('offln_b', (C,), FP32)
    if not h['bv_trivial']:
        din('bv', (128, C), FP32)

    with tile.TileContext(nc) as tc:
        emit(nc, tc, d, out_dram, x2_dram, xnT_dram, h)
    nc.compile()
    return nc


_DECLARED = {'WqT', 'WkT', 'WvT', 'WoT', 'W1T', 'W2T', 'bq', 'bk', 'bo',
             'b1', 'b2', 'dwdiag', 'dwb', 'E8', 'SelW', 'sel2', 'onepm',
             'refy', 'refx', 'wyv', 'wxv', 'id32', 'id16', 'rowi', 'colj'}

_CACHE = {}


def kernel(**inputs):
    h = build_host_consts(inputs)
    if 'nc' not in _CACHE:
        _CACHE['nc'] = build_nc(h)
    nc = _CACHE['nc']

    declared = set(_DECLARED)
    for nm in ('ln1', 'ln2', 'offln'):
        if not h[nm + '_trivial']:
            declared |= {nm + '_g', nm + '_b'}
    if not h['bv_trivial']:
        declared.add('bv')
    shared = {k: v for k, v in h.items()
              if k in declared and isinstance(v, np.ndarray)}

    x = _f32(inputs['x'])
    in_maps = []
    for c in range(NCORES):
        m = dict(shared)
        m['x_shard'] = np.ascontiguousarray(x[c * BL:(c + 1) * BL])
        in_maps.append(m)
    res = run_bass_kernel_spmd(nc, in_maps, list(range(NCORES)))
    outs = [res.results[c]['out'] for c in range(NCORES)]
    return np.concatenate(outs, axis=0).astype(np.float32)



# revision 9
# speedup vs baseline: 1.3929x; 1.3929x over previous
"""Trainium2 Bass kernel for a DAT-style transformer block (sparse_attention).

kernel(**inputs) takes FULL unsharded inputs (B=64), shards the batch across
8 NeuronCores (8 per core, pure data parallel - no collectives), runs one SPMD
Bass/Tile program, returns the FULL [64, 196, 768] float32 output.

v2 design vs the original baseline:
  - A(p) (LN1/q/conv/offset/gather) and B(p-1) (k/v/attn/o/LN2/MLP) are
    emitted INTERLEAVED at chunk granularity via generators, so the in-order
    engine queues keep PE busy on B's matmuls while DVE chews A's vector
    work.  Chunks carry a phase tag (1=exp-table work, 2=gelu-table work)
    and the scheduler never lets the two generators sit in different
    act-table phases, keeping LoadActFuncSet count at ~2/pair.
  - The MLP runs fp8(e4m3) DoubleRow matmuls with weight-split residual
    compensation; weight chunks stream from DRAM in contiguous rows.
  - The 3x3 depthwise conv runs as 9 chained TensorScalarPtr ops on DVE
    (4x bf16 mode) instead of diag-matmuls on PE.
  - All LN rsqrt ops run as Ln+Exp (same act table as softmax Exp).
  - xn^T / img / x2 stay in SBUF (no DRAM spill).
"""

import numpy as np
import ml_dtypes

import concourse.bass as bass
import concourse.mybir as mybir
import concourse.tile as tile
from concourse.bass_utils import run_bass_kernel_spmd

FP32 = mybir.dt.float32
BF16 = mybir.dt.bfloat16
F8 = mybir.dt.float8e4
I16 = mybir.dt.int16
AF = mybir.ActivationFunctionType
ALU = mybir.AluOpType
DR = mybir.MatmulPerfMode.DoubleRow

B = 64
NCORES = 8
BL = 8
N = 196
C = 768
NCH = 6
HEADS = 12
HD = 64
G = 8
CG = 96
MLPD = 3072
MMCH = 24
HH = 14
NCK = [(0, 128), (128, 68)]
EPS = 1e-6
OFF_EPS = 1e-5
NPAD = 208
NTAP = 4
QPW = 290
WSC = 64.0  # power-of-2 scale folded into fp8 MLP weights
MMG = 4     # W1 mm-chunks streamed per DMA


def _f32(x):
    return np.ascontiguousarray(np.asarray(x), dtype=np.float32)


def _bf16(x):
    return np.ascontiguousarray(
        np.asarray(x, dtype=np.float32).astype(ml_dtypes.bfloat16))


def _f8(x):
    return np.ascontiguousarray(
        np.asarray(x, dtype=np.float32).astype(ml_dtypes.float8_e4m3))


def build_host_consts(inp):
    h = {}
    h['WqT'] = _bf16(np.asarray(inp['Wq'], np.float32).T)
    h['WkT'] = _bf16(np.asarray(inp['Wk'], np.float32).T)
    h['WvT'] = _bf16(np.asarray(inp['Wv'], np.float32).T)
    h['WoT'] = _bf16(np.asarray(inp['Wo'], np.float32).T)

    # fp8 MLP weights, DoubleRow k-tile layout, weight-split compensation.
    # W1 stream layout [6, 128, MMG, 3, 2, 128]:
    #   [g4, p, m, i, j, col] = 64*W1[(MMG*g4+m)*128+col, (2i+j)*128+p]
    w1t = (np.asarray(inp['W1'], np.float32).T * WSC)  # [C, MLPD]
    w1r = np.transpose(w1t.reshape(3, 2, 128, MLPD), (2, 0, 1, 3))
    # w1r [p, i, j, mlp] -> [g4, p, m, i, j, col]
    w1s = np.ascontiguousarray(
        np.transpose(w1r.reshape(128, 3, 2, 6, MMG, 128), (3, 0, 4, 1, 2, 5)))
    w1q = np.asarray(_f8(w1s), np.float32)
    h['W1q8'] = _f8(w1s)
    h['W1l8'] = _f8(w1s - w1q)
    # W2 stream layout [NCH, 128, 12, 2, 128]:
    #   [oc, p, r, j, col] = 64*W2[oc*128+col, (2r+j)*128+p]
    w2t = (np.asarray(inp['W2'], np.float32).T * WSC)  # [MLPD, C]
    w2r = np.transpose(w2t.reshape(12, 2, 128, C), (2, 0, 1, 3))
    w2s = np.ascontiguousarray(
        np.transpose(w2r.reshape(128, 12, 2, NCH, 128), (3, 0, 1, 2, 4)))
    w2q = np.asarray(_f8(w2s), np.float32)
    h['W2q8'] = _f8(w2s)
    h['W2l8'] = _f8(w2s - w2q)

    h['bq'] = _f32(np.asarray(inp['bq']).reshape(NCH, 128).T)
    h['bk'] = _f32(np.asarray(inp['bk']).reshape(NCH, 128).T)
    h['bo'] = _f32(np.asarray(inp['bo']).reshape(NCH, 128).T)
    h['b1'] = _f32(np.asarray(inp['b1']).reshape(MMCH, 128).T)
    h['b2'] = _f32(np.asarray(inp['b2']).reshape(NCH, 128).T)

    dw = np.asarray(inp['off_dw_w'], np.float32).reshape(CG, 9)
    dwg = np.tile(dw, (G, 1))  # [C, 9]
    # per-partition conv weights [128, NCH, 9] (fp32 scalar operands)
    h['dww'] = _f32(np.transpose(dwg.reshape(NCH, 128, 9), (1, 0, 2)))
    h['dwb'] = _f32(np.tile(np.asarray(inp['off_dw_b'], np.float32), G)
                    .reshape(NCH, 128).T)

    e8 = np.zeros((G, C), np.float32)
    for c in range(C):
        e8[c // CG, c] = 1.0
    h['E8'] = _bf16(e8)

    sel = np.zeros((2, 128, 13 * 128), np.float32)
    for i, (off, nsz) in enumerate(NCK):
        for nl in range(nsz):
            f, p16 = divmod(off + nl, 16)
            for band in range(8):
                sel[i, nl, f * 128 + band * 16 + p16] = 1.0
    h['SelW'] = _bf16(sel)

    sel2 = np.zeros((2, 128), np.float32)
    sel2[0, 0:64] = 1.0
    sel2[1, 64:128] = 1.0
    h['sel2'] = _bf16(sel2)

    h['onepm'] = _bf16(np.tile(np.array([1.0, 0.0, 0.0, 1.0], np.float32),
                               (128, 1)))

    ii = np.arange(HH, dtype=np.float32)
    h['refy'] = _f32(np.repeat((ii + 0.5) * 13.0 / 14.0, HH))
    h['refx'] = _f32(np.tile((ii + 0.5) * 13.0 / 14.0, HH))
    h['rowi'] = _f32(np.repeat(ii, HH))
    h['colj'] = _f32(np.tile(ii, HH))

    pw = np.asarray(inp['off_proj_w'], np.float32)
    h['wyv'] = _bf16(np.tile(pw[0], G))
    h['wxv'] = _bf16(np.tile(pw[1], G))

    h['id16'] = _bf16(np.eye(128, dtype=np.float32))
    h['id8'] = _f8(np.eye(128, dtype=np.float32))

    for nm, gk, bk_ in (('ln1', 'ln1_g', 'ln1_b'), ('ln2', 'ln2_g', 'ln2_b')):
        g = np.asarray(inp[gk], np.float32)
        bb = np.asarray(inp[bk_], np.float32)
        h[nm + '_trivial'] = bool(np.all(g == 1.0) and np.all(bb == 0.0))
        h[nm + '_g'] = _f32(g)
        h[nm + '_b'] = _f32(bb)
    og = np.tile(np.asarray(inp['off_ln_g'], np.float32), G)
    ob = np.tile(np.asarray(inp['off_ln_b'], np.float32), G)
    h['offln_trivial'] = bool(np.all(og == 1.0) and np.all(ob == 0.0))
    h['offln_g'] = _f32(og)
    h['offln_b'] = _f32(ob)
    bv = np.asarray(inp['bv'], np.float32)
    h['bv_trivial'] = bool(np.all(bv == 0.0))
    h['bv'] = _f32(np.tile(bv.reshape(1, C), (128, 1)))
    return h


def _free_bcast(t_ap, inner):
    """View [P, F] AP as [P, F, inner] with a stride-0 inner dim."""
    return bass.AP(tensor=t_ap.tensor, offset=t_ap.offset,
                   ap=list(t_ap.ap) + [[0, inner]])


def _dram_bcast(src_ap, rows):
    return bass.AP(tensor=src_ap.tensor, offset=src_ap.offset,
                   ap=[[0, rows]] + list(src_ap.ap))


def emit(nc, tc, d, out_dram, h):
    x_in = d['x_shard']

    with (
        tc.tile_pool(name='cw', bufs=1) as cw,
        tc.tile_pool(name='pair', bufs=2) as pp,
        tc.tile_pool(name='wstream', bufs=2) as ws,
        tc.tile_pool(name='tmp', bufs=3) as tp,
        tc.tile_pool(name='tbig', bufs=2) as tb,
        tc.tile_pool(name='p_sm', bufs=2) as sm,
        tc.tile_pool(name='ps_mm', bufs=5, space='PSUM') as psm,
        tc.tile_pool(name='ps_tp', bufs=3, space='PSUM') as pst,
    ):
        # ---- resident constants --------------------------------------
        wqkvo = {}
        for wn in ('WqT', 'WkT', 'WvT', 'WoT'):
            wt = cw.tile([128, NCH, C], BF16, tag=wn, name=wn)
            nc.sync.dma_start(
                out=wt[:],
                in_=d[wn][:].rearrange('(k p) c -> p k c', k=NCH))
            wqkvo[wn] = wt
        WqT = [wqkvo['WqT'][:, k, :] for k in range(NCH)]
        WkT = [wqkvo['WkT'][:, k, :] for k in range(NCH)]
        WvT = [wqkvo['WvT'][:, k, :] for k in range(NCH)]
        WoT = [wqkvo['WoT'][:, k, :] for k in range(NCH)]

        dww = cw.tile([128, NCH, 9], FP32, tag='dww', name='dww')
        nc.sync.dma_start(out=dww[:], in_=d['dww'][:])

        E8 = cw.tile([G, C], BF16, tag='e8', name='e8')
        nc.sync.dma_start(out=E8[:], in_=d['E8'][:])
        sel2_t = cw.tile([2, 128], BF16, tag='sel2', name='sel2')
        nc.sync.dma_start(out=sel2_t[:], in_=d['sel2'][:])
        onepm_t = cw.tile([128, 4], BF16, tag='onepm', name='onepm')
        nc.sync.dma_start(out=onepm_t[:], in_=d['onepm'][:])
        SelW = [cw.tile([128, 13 * 128], BF16, tag=f'sel{i}', name=f'sel{i}')
                for i in range(2)]
        for i in range(2):
            nc.sync.dma_start(out=SelW[i][:], in_=d['SelW'][i])
        id16 = cw.tile([128, 128], BF16, tag='id16', name='id16')
        nc.sync.dma_start(out=id16[:], in_=d['id16'][:])
        id8 = cw.tile([128, 128], F8, tag='id8', name='id8')
        nc.sync.dma_start(out=id8[:], in_=d['id8'][:])
        bias_t = {}
        for nm, cols in (('bq', NCH), ('bk', NCH), ('bo', NCH), ('b1', MMCH),
                         ('b2', NCH), ('dwb', NCH)):
            bias_t[nm] = cw.tile([128, cols], FP32, tag='bias_' + nm,
                                 name='bias_' + nm)
            nc.sync.dma_start(out=bias_t[nm][:], in_=d[nm][:])
        refy_t, refx_t, rowi_t, colj_t = [], [], [], []
        for i, (off, nsz) in enumerate(NCK):
            for nm, lst in (('refy', refy_t), ('refx', refx_t),
                            ('rowi', rowi_t), ('colj', colj_t)):
                tt = cw.tile([nsz, 1], FP32, tag=f'{nm}{i}', name=f'{nm}{i}')
                nc.sync.dma_start(
                    out=tt[:],
                    in_=d[nm][off:off + nsz].rearrange('(n one) -> n one',
                                                       one=1))
                lst.append(tt)
        wyb = cw.tile([128, C], BF16, tag='wyb', name='wyb')
        wxb = cw.tile([128, C], BF16, tag='wxb', name='wxb')
        nc.sync.dma_start(out=wyb[:], in_=_dram_bcast(d['wyv'][:], 128))
        nc.sync.dma_start(out=wxb[:], in_=_dram_bcast(d['wxv'][:], 128))
        eps_t = cw.tile([128, 1], FP32, tag='eps', name='eps')
        nc.vector.memset(eps_t[:], EPS)
        oeps_t = cw.tile([128, 1], FP32, tag='oeps', name='oeps')
        nc.vector.memset(oeps_t[:], OFF_EPS)
        gbt = {}
        for nm in ('ln1', 'ln2', 'offln'):
            if not h[nm + '_trivial']:
                g_ = cw.tile([128, C], FP32, tag=nm + 'g', name=nm + 'g')
                b_ = cw.tile([128, C], FP32, tag=nm + 'b', name=nm + 'b')
                nc.sync.dma_start(out=g_[:], in_=_dram_bcast(d[nm + '_g'][:], 128))
                nc.sync.dma_start(out=b_[:], in_=_dram_bcast(d[nm + '_b'][:], 128))
                gbt[nm] = (g_, b_)
        bv_t = None
        if not h['bv_trivial']:
            bv_t = cw.tile([128, C], FP32, tag='bvt', name='bvt')
            nc.sync.dma_start(out=bv_t[:], in_=d['bv'][:])

        def rstd_of(var_ap, nsz, eps_tile, cols=1):
            """1/sqrt(var+eps) via Ln+Exp (stays in the exp act table)."""
            lnv = sm.tile([128, cols], FP32, tag='ln_lnv', name='ln_lnv')
            nc.scalar.activation(out=lnv[:nsz], in_=var_ap, func=AF.Ln,
                                 bias=eps_tile[:nsz], scale=1.0)
            rstd = sm.tile([128, cols], FP32, tag='ln_rstd', name='ln_rstd')
            nc.scalar.activation(out=rstd[:nsz], in_=lnv[:nsz], func=AF.Exp,
                                 scale=-0.5)
            return rstd

        def ln_norm(xf, nsz, out_ap, gbk):
            st = sm.tile([128, 3, 6], FP32, tag='ln_st', name='ln_st')
            for s in range(3):
                nc.vector.bn_stats(out=st[:nsz, s, :],
                                   in_=xf[:nsz, s * 256:(s + 1) * 256])
            mv = sm.tile([128, 2], FP32, tag='ln_mv', name='ln_mv')
            nc.vector.bn_aggr(out=mv[:nsz], in_=st[:nsz])
            rstd = rstd_of(mv[:nsz, 1:2], nsz, eps_t)
            nmr = sm.tile([128, 1], FP32, tag='ln_nmr', name='ln_nmr')
            nc.vector.tensor_scalar(out=nmr[:nsz], in0=mv[:nsz, 0:1],
                                    scalar1=rstd[:nsz], scalar2=-1.0,
                                    op0=ALU.mult, op1=ALU.mult)
            if gbk is None:
                nc.scalar.activation(out=out_ap, in_=xf[:nsz], func=AF.Identity,
                                     bias=nmr[:nsz], scale=rstd[:nsz])
            else:
                gt, bt = gbk
                tmp = tb.tile([128, C], FP32, tag='xb', name='ln_tmp')
                nc.scalar.activation(out=tmp[:nsz], in_=xf[:nsz], func=AF.Identity,
                                     bias=nmr[:nsz], scale=rstd[:nsz])
                nc.vector.tensor_mul(out=tmp[:nsz], in0=tmp[:nsz], in1=gt[:nsz])
                nc.vector.tensor_add(out=out_ap, in0=tmp[:nsz], in1=bt[:nsz])

        # per-pair state handed from A(p) to B(p)
        state = {}

        def q_mov(qp_p, hp, bi, p0, np_=64):
            """[np_, 14, 14] moving-operand view of the padded q layout."""
            base = qp_p[hp][p0:p0 + np_, bi * QPW + 17:bi * QPW + 18]
            return bass.AP(tensor=base.tensor, offset=base.offset,
                           ap=[base.ap[0], [16, 14], [1, 14]])

        def stA(p):
            bp = 2 * p
            # ---------------- ST1: LN1 + transposes -------------------
            xnTb = [pp.tile([128, 2, N], BF16, tag=f'xnTb{k}',
                            name=f'xnTb{k}', bufs=1) for k in range(NCH)]
            imgs = pp.tile([128, NCH, 2, N], FP32, tag='img', name='img',
                           bufs=1)
            for bi, b in enumerate((bp, bp + 1)):
                for i, (off, nsz) in enumerate(NCK):
                    yield 1
                    xf = tb.tile([128, C], FP32, tag='xa', name='st1_x', bufs=2)
                    nc.sync.dma_start(out=xf[:nsz], in_=x_in[b, off:off + nsz, :])
                    xn = tb.tile([128, C], BF16, tag='xn', name='st1_xn',
                                 bufs=2)
                    ln_norm(xf, nsz, xn[:nsz], gbt.get('ln1'))
                    for cc in range(NCH):
                        pt = pst.tile([128, 128], BF16, tag='tp',
                                      name='st1_ps')
                        nc.tensor.transpose(
                            pt[:, :nsz], xn[:nsz, cc * 128:(cc + 1) * 128],
                            id16[:nsz, :nsz])
                        nc.vector.tensor_copy(
                            out=xnTb[cc][:, bi, off:off + nsz],
                            in_=pt[:, :nsz])
                        nc.scalar.activation(
                            out=imgs[:, cc, bi, off:off + nsz],
                            in_=pt[:, :nsz], func=AF.Identity)

            # ---------------- ST2: q ----------------------------------
            qp_p = [pp.tile([128, 2 * QPW], BF16, tag=f'qp{k}',
                            name=f'qp{k}', bufs=2) for k in range(NCH)]
            if p < 2:
                for k in range(NCH):
                    nc.gpsimd.memset(qp_p[k][:], 0.0)
            for oc in range(NCH):
                yield 1
                q_ps = psm.tile([128, 392], FP32, tag='mm', name='st2_ps')
                for kc in range(NCH):
                    nc.tensor.matmul(
                        q_ps[:], WqT[kc][:, oc * 128:(oc + 1) * 128],
                        xnTb[kc][:], start=(kc == 0), stop=(kc == NCH - 1))
                for bi in range(2):
                    base = qp_p[oc][:, bi * QPW + 17:bi * QPW + 18]
                    outap = bass.AP(tensor=base.tensor, offset=base.offset,
                                    ap=[base.ap[0], [16, 14], [1, 14]])
                    nc.scalar.activation(
                        out=outap, in_=q_ps[:, bi * N:(bi + 1) * N],
                        func=AF.Identity, bias=bias_t['bq'][:, oc:oc + 1])

            # ------- ST3: depthwise conv (DVE TensorScalarPtr chain) --
            ocT = {}
            for bi, b in enumerate((bp, bp + 1)):
                ocT[b] = tp.tile([128, 2, C], BF16, tag='st3_ocT',
                                 name='st3_ocT', bufs=2)
            for oc in range(NCH):
                yield 1
                cv = tp.tile([128, 2, 256], BF16, tag='st3_cv',
                             name='st3_cv', bufs=2)

                def win(t):
                    ky, kx = divmod(t, 3)
                    base = qp_p[oc][:, 0:1]
                    return bass.AP(tensor=base.tensor,
                                   offset=base.offset + 16 * ky + kx,
                                   ap=[base.ap[0], [QPW, 2], [1, 256]])

                nc.vector.tensor_scalar(out=cv[:], in0=win(0),
                                        scalar1=dww[:, oc, 0:1],
                                        scalar2=None, op0=ALU.mult)
                for t in range(1, 9):
                    nc.vector.scalar_tensor_tensor(
                        out=cv[:], in0=win(t), scalar=dww[:, oc, t:t + 1],
                        in1=cv[:], op0=ALU.mult, op1=ALU.add)
                for bi, b in enumerate((bp, bp + 1)):
                    cvb = tp.tile([128, N], BF16, tag='st3_cvb',
                                  name='st3_cvb', bufs=2)
                    base = cv[:, bi, 0:1]
                    inap = bass.AP(tensor=base.tensor, offset=base.offset,
                                   ap=[base.ap[0], [16, 14], [1, 14]])
                    nc.scalar.activation(out=cvb[:], in_=inap,
                                         func=AF.Identity,
                                         bias=bias_t['dwb'][:, oc:oc + 1])
                    for i, (off, nsz) in enumerate(NCK):
                        pt = pst.tile([128, 128], BF16, tag='tp',
                                      name='st3_tp')
                        nc.tensor.transpose(pt[:nsz, :], cvb[:, off:off + nsz],
                                            id16)
                        nc.vector.tensor_copy(
                            out=ocT[b][:nsz, i, oc * 128:(oc + 1) * 128],
                            in_=pt[:nsz, :])

            # ---------------- ST4 pass 1: group-LN (exp table) --------
            ogs = {}
            for bi, b in enumerate((bp, bp + 1)):
                for i, (off, nsz) in enumerate(NCK):
                    yield 1
                    sl = ocT[b][:nsz, i, :]
                    st8 = sm.tile([128, G, 6], FP32, tag='off_st',
                                  name='off_st')
                    mv8 = sm.tile([128, G, 2], FP32, tag='off_mv',
                                  name='off_mv')
                    for g in range(G):
                        nc.vector.bn_stats(out=st8[:nsz, g, :],
                                           in_=sl[:, g * CG:(g + 1) * CG])
                        nc.vector.bn_aggr(out=mv8[:nsz, g, :],
                                          in_=st8[:nsz, g, :])
                    rec8 = rstd_of(mv8[:nsz, :, 1], nsz, oeps_t, cols=G)
                    og = tp.tile([128, C], BF16, tag='off_og', name='off_og',
                                 bufs=4)
                    ogv = og[:nsz].rearrange('p (g c) -> p g c', g=G)
                    nc.gpsimd.tensor_tensor(
                        out=ogv, in0=sl.rearrange('p (g c) -> p g c', g=G),
                        in1=_free_bcast(mv8[:nsz, :, 0], CG),
                        op=ALU.subtract)
                    nc.vector.tensor_tensor(out=ogv, in0=ogv,
                                            in1=_free_bcast(rec8[:nsz], CG),
                                            op=ALU.mult)
                    if not h['offln_trivial']:
                        gt, bt = gbt['offln']
                        nc.vector.tensor_mul(out=og[:nsz], in0=og[:nsz],
                                             in1=gt[:nsz])
                        nc.vector.tensor_add(out=og[:nsz], in0=og[:nsz],
                                             in1=bt[:nsz])
                    ogs[(bi, i)] = og

            # ---------------- ST4 pass 2: gelu/tanh (gelu table) ------
            idx4p = [sm.tile([128, 2, NTAP, G], BF16, tag=f'idx4_{i}',
                             name=f'idx4_{i}', bufs=2) for i in range(2)]
            W48 = {}
            for bi, b in enumerate((bp, bp + 1)):
                W48[b] = sm.tile([G, NTAP * NPAD], BF16, tag='w48',
                                 name='w48', bufs=2)
                if p < 2:
                    base = W48[b][:, 0:1]
                    padap = bass.AP(tensor=base.tensor,
                                    offset=base.offset + N,
                                    ap=[base.ap[0], [NPAD, NTAP],
                                        [1, NPAD - N]])
                    nc.vector.memset(padap, 0.0)
                for i, (off, nsz) in enumerate(NCK):
                    yield 2
                    og = ogs.pop((bi, i))
                    nc.scalar.activation(out=og[:nsz], in_=og[:nsz],
                                         func=AF.Gelu)
                    oyx = sm.tile([128, 16], FP32, tag='off_oyx',
                                  name='off_oyx')
                    tpm = tp.tile([128, C], BF16, tag='off_tpm',
                                  name='off_tpm', bufs=2)
                    nc.vector.tensor_mul(out=tpm[:nsz], in0=og[:nsz],
                                         in1=wyb[:nsz])
                    nc.vector.tensor_reduce(
                        out=oyx[:nsz, 0:G],
                        in_=tpm[:nsz].rearrange('p (g c) -> p g c', g=G),
                        axis=mybir.AxisListType.X, op=ALU.add)
                    nc.vector.tensor_mul(out=tpm[:nsz], in0=og[:nsz],
                                         in1=wxb[:nsz])
                    nc.vector.tensor_reduce(
                        out=oyx[:nsz, G:16],
                        in_=tpm[:nsz].rearrange('p (g c) -> p g c', g=G),
                        axis=mybir.AxisListType.X, op=ALU.add)
                    th = sm.tile([128, 16], FP32, tag='off_th', name='off_th')
                    nc.scalar.activation(out=th[:nsz], in_=oyx[:nsz],
                                         func=AF.Tanh)
                    gy = sm.tile([128, G], FP32, tag='off_gy', name='off_gy')
                    gx = sm.tile([128, G], FP32, tag='off_gx', name='off_gx')
                    nc.vector.tensor_scalar(out=gy[:nsz], in0=th[:nsz, 0:G],
                                            scalar1=6.5 / 14.0,
                                            scalar2=refy_t[i][:],
                                            op0=ALU.mult, op1=ALU.add)
                    nc.vector.tensor_scalar(out=gx[:nsz], in0=th[:nsz, G:16],
                                            scalar1=6.5 / 14.0,
                                            scalar2=refx_t[i][:],
                                            op0=ALU.mult, op1=ALU.add)
                    fy = sm.tile([128, G], FP32, tag='off_fy', name='off_fy')
                    fx = sm.tile([128, G], FP32, tag='off_fx', name='off_fx')
                    y0 = sm.tile([128, G], FP32, tag='off_y0', name='off_y0')
                    x0 = sm.tile([128, G], FP32, tag='off_x0', name='off_x0')
                    # floor(gy) = rowi - [gy < rowi]  (exact: |offset|<0.5px)
                    nc.vector.tensor_scalar(out=y0[:nsz], in0=gy[:nsz],
                                            scalar1=rowi_t[i][:], scalar2=None,
                                            op0=ALU.is_lt)
                    nc.vector.tensor_scalar(out=y0[:nsz], in0=y0[:nsz],
                                            scalar1=-1.0, scalar2=rowi_t[i][:],
                                            op0=ALU.mult, op1=ALU.add)
                    nc.vector.tensor_scalar(out=x0[:nsz], in0=gx[:nsz],
                                            scalar1=colj_t[i][:], scalar2=None,
                                            op0=ALU.is_lt)
                    nc.vector.tensor_scalar(out=x0[:nsz], in0=x0[:nsz],
                                            scalar1=-1.0, scalar2=colj_t[i][:],
                                            op0=ALU.mult, op1=ALU.add)
                    nc.vector.tensor_scalar_min(out=y0[:nsz], in0=y0[:nsz],
                                                scalar1=12.0)
                    nc.vector.tensor_scalar_min(out=x0[:nsz], in0=x0[:nsz],
                                                scalar1=12.0)
                    nc.vector.tensor_sub(out=fy[:nsz], in0=gy[:nsz],
                                         in1=y0[:nsz])
                    nc.vector.tensor_sub(out=fx[:nsz], in0=gx[:nsz],
                                         in1=x0[:nsz])
                    ia = sm.tile([128, G], FP32, tag='off_ia', name='off_ia')
                    nc.vector.scalar_tensor_tensor(out=ia[:nsz], in0=y0[:nsz],
                                                   scalar=14.0, in1=x0[:nsz],
                                                   op0=ALU.mult, op1=ALU.add)
                    nc.vector.tensor_copy(out=idx4p[i][:nsz, bi, 0, :],
                                          in_=ia[:nsz])
                    for t, ofs in ((1, 14.0), (2, 1.0), (3, 15.0)):
                        nc.vector.tensor_scalar_add(
                            out=idx4p[i][:nsz, bi, t, :], in0=ia[:nsz],
                            scalar1=ofs)
                    fy1 = sm.tile([128, G], FP32, tag='off_fy1',
                                  name='off_fy1')
                    fx1 = sm.tile([128, G], FP32, tag='off_fx1',
                                  name='off_fx1')
                    nc.vector.tensor_scalar(out=fy1[:nsz], in0=fy[:nsz],
                                            scalar1=-1.0, scalar2=1.0,
                                            op0=ALU.mult, op1=ALU.add)
                    nc.vector.tensor_scalar(out=fx1[:nsz], in0=fx[:nsz],
                                            scalar1=-1.0, scalar2=1.0,
                                            op0=ALU.mult, op1=ALU.add)
                    for t, (aa, bb2) in enumerate(((fx1, fy1), (fx1, fy),
                                                   (fx, fy1), (fx, fy))):
                        wt = sm.tile([128, G], BF16, tag='off_wt',
                                     name='off_wt')
                        nc.vector.tensor_mul(out=wt[:nsz], in0=aa[:nsz],
                                             in1=bb2[:nsz])
                        ptw = pst.tile([G, 128], BF16, tag='tp',
                                       name='off_ptw')
                        nc.tensor.transpose(ptw[:, :nsz], wt[:nsz],
                                            id16[:nsz, :nsz])
                        nc.vector.tensor_copy(
                            out=W48[b][:, t * NPAD + off:t * NPAD + off + nsz],
                            in_=ptw[:, :nsz])

            yield 2
            # wrap indices into gather layout (split across two PSUM tiles)
            wrapP = [psm.tile([128, 7 * 64], FP32, tag='mm', name='st4_wrap')
                     for _ in range(2)]
            for f in range(13):
                w_t, fo = (wrapP[0], f) if f < 7 else (wrapP[1], f - 7)
                for i in range(2):
                    nsz = NCK[i][1]
                    nc.tensor.matmul(
                        w_t[:, fo * 64:(fo + 1) * 64],
                        SelW[i][:nsz, f * 128:(f + 1) * 128],
                        idx4p[i][:nsz].rearrange('p b t g -> p (b t g)'),
                        start=(i == 0), stop=(i == 1))
            wrapS = sm.tile([128, G, 2, NTAP, 13], I16, tag='wrapS',
                            name='wrapS', bufs=2)
            for bb in range(2):
                for half, (f0, fn) in enumerate(((0, 7), (7, 6))):
                    base_in = wrapP[half][:, 0:1]
                    inap = bass.AP(tensor=base_in.tensor,
                                   offset=base_in.offset + bb * 32,
                                   ap=[base_in.ap[0], [64, fn], [8, NTAP],
                                       [1, G]])
                    base_out = wrapS[:, 0, bb, 0, f0:f0 + 1]
                    outap = bass.AP(tensor=base_out.tensor,
                                    offset=base_out.offset,
                                    ap=[base_out.ap[0], [1, fn], [13, NTAP],
                                        [104, G]])
                    nc.vector.tensor_copy(out=outap, in_=inap)
            idxt = [sm.tile([128, 2, NTAP, 13], I16, tag=f'idxt{j}',
                            name=f'idxt{j}', bufs=2) for j in range(NCH)]
            for j in range(NCH):
                bands_g = [(8 * j + band) // 6 for band in range(8)]
                runs = []
                r0 = 0
                for band in range(1, 9):
                    if band == 8 or bands_g[band] != bands_g[r0]:
                        runs.append((r0, band - 1, bands_g[r0]))
                        r0 = band
                for (b0, b1, g) in runs:
                    p0, pn = 16 * b0, 16 * (b1 - b0 + 1)
                    nc.sync.dma_start(out=idxt[j][p0:p0 + pn],
                                      in_=wrapS[p0:p0 + pn, g])

            # ---------------- ST5: gather + bilinear weighting --------
            sampled = [pp.tile([128, 2, N], BF16, tag=f'smp{k}',
                               name=f'smp{k}', bufs=2) for k in range(NCH)]
            for bi, b in enumerate((bp, bp + 1)):
                for j in range(NCH):
                    yield 2
                    gws = []
                    for half in range(2):
                        gth = tp.tile([128, 2 * NPAD], FP32, tag='st5_g',
                                      name='st5_g', bufs=2)
                        nc.gpsimd.ap_gather(
                            out_ap=gth[:],
                            in_ap=imgs[:, j, bi, :].rearrange(
                                'p (n one) -> p n one', one=1),
                            idxs_ap=idxt[j][:, bi, 2 * half:2 * half + 2, :]
                            .rearrange('p t f -> p (t f)'),
                            channels=128, num_elems=N, d=1,
                            num_idxs=2 * NPAD)
                        wb = psm.tile([128, 2 * NPAD], FP32, tag='mm',
                                      name=f'st5_w{half}')
                        nc.tensor.matmul(
                            wb[:], E8[:, j * 128:(j + 1) * 128],
                            W48[b][:, half * 2 * NPAD:(half + 1) * 2 * NPAD],
                            start=True, stop=True)
                        gw = tp.tile([128, 2 * NPAD], BF16, tag='st5_gw',
                                     name='st5_gw', bufs=2)
                        nc.vector.tensor_tensor(out=gw[:], in0=gth[:],
                                                in1=wb[:], op=ALU.mult)
                        gws.append(gw)
                    s01 = tp.tile([128, N], BF16, tag='st5_s01',
                                  name='st5_s01', bufs=2)
                    nc.vector.tensor_add(out=s01[:], in0=gws[0][:, 0:N],
                                         in1=gws[0][:, NPAD:NPAD + N])
                    s23 = tp.tile([128, N], BF16, tag='st5_s23',
                                  name='st5_s23', bufs=2)
                    nc.vector.tensor_add(out=s23[:], in0=gws[1][:, 0:N],
                                         in1=gws[1][:, NPAD:NPAD + N])
                    nc.vector.tensor_add(out=sampled[j][:, bi, :],
                                         in0=s01[:], in1=s23[:])
            state[p] = (qp_p, sampled)

        def stB(p):
            bp = 2 * p
            qp_p, sampled = state.pop(p)
            # ---------------- ST6: k and vT ---------------------------
            k_p = [pp.tile([128, 2, N], BF16, tag=f'kk{k}', name=f'kk{k}',
                           bufs=1) for k in range(NCH)]
            vT_p = pp.tile([128, 2, 2, C], BF16, tag='vT', name='vT', bufs=1)
            for oc in range(NCH):
                yield 1
                k_ps = psm.tile([128, 392], FP32, tag='mm', name='st6_kps')
                for kc in range(NCH):
                    nc.tensor.matmul(
                        k_ps[:], WkT[kc][:, oc * 128:(oc + 1) * 128],
                        sampled[kc][:], start=(kc == 0), stop=(kc == NCH - 1))
                nc.scalar.activation(
                    out=k_p[oc][:], in_=k_ps[:].rearrange('p (b n) -> p b n',
                                                          b=2),
                    func=AF.Identity, bias=bias_t['bk'][:, oc:oc + 1])
            for bi in range(2):
                for i, (off, nsz) in enumerate(NCK):
                    yield 1
                    for half in range(2):
                        v_ps = psm.tile([128, 384], FP32, tag='mm',
                                        name='st6_vps')
                        for kc in range(NCH):
                            nc.tensor.matmul(
                                v_ps[:nsz], sampled[kc][:, bi, off:off + nsz],
                                WvT[kc][:, half * 384:(half + 1) * 384],
                                start=(kc == 0), stop=(kc == NCH - 1))
                        dst = vT_p[:nsz, bi, i, half * 384:(half + 1) * 384]
                        if bv_t is None:
                            nc.vector.tensor_copy(out=dst, in_=v_ps[:nsz])
                        else:
                            nc.vector.tensor_add(
                                out=dst, in0=v_ps[:nsz],
                                in1=bv_t[:nsz, half * 384:(half + 1) * 384])

            # ---------------- ST7: attention (S^T form) ---------------
            aop = pp.tile([128, NCH, 2, N], BF16, tag='st7_ao',
                          name='st7_ao', bufs=1)
            for bi, b in enumerate((bp, bp + 1)):
                for hp in range(NCH):
                    yield 1
                    o_ps = psm.tile([128, N], FP32, tag='mm', name='st7_ops')
                    sums_ps = pst.tile([2, N], FP32, tag='tp',
                                       name='st7_sums')
                    for hh in range(2):
                        hd = hp * 2 + hh
                        p0 = hh * 64
                        expT = tp.tile([128, 2, N], BF16, tag='st7_exp',
                                       name='st7_exp', bufs=2)
                        s_ps = psm.tile([128, 2, N], FP32, tag='mm',
                                        name='st7_sps')
                        for ni, (noff, nsz) in enumerate(NCK):
                            nc.tensor.matmul(
                                s_ps[:nsz, ni, :],
                                k_p[hp][p0:p0 + 64, bi, noff:noff + nsz],
                                q_mov(qp_p, hp, bi, p0),
                                start=True, stop=True)
                        nc.scalar.activation(out=expT[:], in_=s_ps[:],
                                             func=AF.Exp, scale=0.125)
                        for ni, (noff, nsz) in enumerate(NCK):
                            nc.tensor.matmul(
                                o_ps[p0:p0 + 64, :],
                                vT_p[:nsz, bi, ni, hd * 64:(hd + 1) * 64],
                                expT[:nsz, ni, :],
                                start=(ni == 0), stop=(ni == 1))
                            nc.tensor.matmul(
                                sums_ps[:],
                                onepm_t[:nsz, 2 * hh:2 * hh + 2],
                                expT[:nsz, ni, :],
                                start=(hh == 0 and ni == 0),
                                stop=(hh == 1 and ni == 1))
                    rec2 = sm.tile([2, N], BF16, tag='st7_rec',
                                   name='st7_rec', bufs=4)
                    with nc.allow_low_precision(
                            reason='softmax recip to bf16 for PE bcast; '
                                   '2e-2 tol'):
                        nc.vector.reciprocal(out=rec2[:], in_=sums_ps[:])
                    bc_ps = psm.tile([128, N], FP32, tag='mm', name='st7_bc')
                    nc.tensor.matmul(bc_ps[:], sel2_t[:], rec2[:],
                                     start=True, stop=True)
                    bc_sb = tp.tile([128, N], BF16, tag='st7_bcs',
                                    name='st7_bcs', bufs=2)
                    nc.scalar.activation(out=bc_sb[:], in_=bc_ps[:],
                                         func=AF.Identity)
                    nc.vector.tensor_tensor(out=aop[:, hp, bi, :],
                                            in0=o_ps[:], in1=bc_sb[:],
                                            op=ALU.mult)

            # o-proj, residual 1, LN2, transposes
            xn2T = pp.tile([128, 3, 2, 2 * N], F8, tag='x2T', name='x2T',
                           bufs=1)
            x2sb = pp.tile([128, 2, 2, C], BF16, tag='x2sb', name='x2sb',
                           bufs=1)
            ybf = pp.tile([128, NCH, 2, N], BF16, tag='ybm2',
                          name='st7_ybf', bufs=1)
            for oc in range(NCH):
                yield 1
                y_ps = psm.tile([128, 392], FP32, tag='mm', name='st7_yps')
                for kc in range(NCH):
                    nc.tensor.matmul(
                        y_ps[:], WoT[kc][:, oc * 128:(oc + 1) * 128],
                        aop[:, kc, :, :], start=(kc == 0),
                        stop=(kc == NCH - 1))
                nc.scalar.activation(
                    out=ybf[:, oc, :, :],
                    in_=y_ps[:].rearrange('p (b n) -> p b n', b=2),
                    func=AF.Identity, bias=bias_t['bo'][:, oc:oc + 1])
            for bi, b in enumerate((bp, bp + 1)):
                for i, (off, nsz) in enumerate(NCK):
                    yield 1
                    xo = tb.tile([128, C], FP32, tag='xa', name='st7_xo', bufs=2)
                    nc.sync.dma_start(out=xo[:nsz], in_=x_in[b, off:off + nsz, :])
                    for oc in range(NCH):
                        ypt = pst.tile([128, 128], BF16, tag='tp',
                                       name='st7_ypt')
                        nc.tensor.transpose(ypt[:nsz, :],
                                            ybf[:, oc, bi, off:off + nsz],
                                            id16)
                        nc.vector.tensor_add(
                            out=x2sb[:nsz, bi, i, oc * 128:(oc + 1) * 128],
                            in0=ypt[:nsz, :],
                            in1=xo[:nsz, oc * 128:(oc + 1) * 128])
                    xn2 = tb.tile([128, C], BF16, tag='xn2', name='st7_xn2',
                                  bufs=2)
                    ln_norm(x2sb[:, bi, i, :], nsz, xn2[:nsz],
                            gbt.get('ln2'))
                    for cc in range(NCH):
                        pt = pst.tile([128, 128], BF16, tag='tp',
                                      name='st7_tps')
                        nc.tensor.transpose(
                            pt[:, :nsz], xn2[:nsz, cc * 128:(cc + 1) * 128],
                            id16[:nsz, :nsz])
                        nc.vector.tensor_copy(
                            out=xn2T[:, cc >> 1, cc & 1,
                                     bi * N + off:bi * N + off + nsz],
                            in_=pt[:, :nsz])

            # ---------------- ST9: MLP (fp8 DoubleRow) + residual 2 ---
            h1_all = tp.tile([128, 12, 2, 392], F8, tag='h1_all',
                             name='h1_all', bufs=1)
            for g4 in range(6):
                w1q = ws.tile([128, MMG, 3, 2, 128], F8, tag='w1cq',
                              name='w1cq', bufs=2)
                nc.sync.dma_start(out=w1q[:], in_=d['W1q8'][g4])
                w1l = ws.tile([128, MMG, 3, 2, 128], F8, tag='w1cl',
                              name='w1cl', bufs=2)
                nc.sync.dma_start(out=w1l[:], in_=d['W1l8'][g4])
                for m4 in range(MMG):
                    mm = g4 * MMG + m4
                    yield 2
                    h1_ps = psm.tile([128, 392], FP32, tag='mm', name='h1ps')
                    for i in range(3):
                        for wi, wt in enumerate((w1q, w1l)):
                            nc.tensor.matmul(
                                h1_ps[:], wt[:, m4, i, :, :],
                                xn2T[:, i, :, :],
                                start=(i == 0 and wi == 0),
                                stop=(i == 2 and wi == 1),
                                perf_mode=DR)
                    nc.scalar.activation(out=h1_all[:, mm >> 1, mm & 1, :],
                                         in_=h1_ps[:], func=AF.Gelu,
                                         bias=bias_t['b1'][:, mm:mm + 1],
                                         scale=1.0 / WSC)
            m2b = pp.tile([128, NCH, 2, N], BF16, tag='ybm2',
                          name='st9_m2b', bufs=1)
            for oc in range(NCH):
                w2q = ws.tile([128, 12, 2, 128], F8, tag='w2cq',
                              name='w2cq', bufs=2)
                nc.sync.dma_start(out=w2q[:], in_=d['W2q8'][oc])
                w2l = ws.tile([128, 12, 2, 128], F8, tag='w2cl',
                              name='w2cl', bufs=2)
                nc.sync.dma_start(out=w2l[:], in_=d['W2l8'][oc])
                yield 2
                m2_ps = psm.tile([128, 392], FP32, tag='mm', name='m2ps')
                for r in range(12):
                    for wi, wt in enumerate((w2q, w2l)):
                        nc.tensor.matmul(
                            m2_ps[:], wt[:, r, :, :],
                            h1_all[:, r, :, :],
                            start=(r == 0 and wi == 0),
                            stop=(r == 11 and wi == 1),
                            perf_mode=DR)
                nc.scalar.activation(
                    out=m2b[:, oc, :, :],
                    in_=m2_ps[:].rearrange('p (b n) -> p b n', b=2),
                    func=AF.Identity, bias=bias_t['b2'][:, oc:oc + 1],
                    scale=1.0 / WSC)
            for bi, b in enumerate((bp, bp + 1)):
                for i, (off, nsz) in enumerate(NCK):
                    yield 2
                    ot = tb.tile([128, C], FP32, tag='xb', name='st9_out',
                                 bufs=2)
                    for oc in range(NCH):
                        mpt = pst.tile([128, 128], BF16, tag='tp',
                                       name='st9_mpt')
                        nc.tensor.transpose(mpt[:nsz, :],
                                            m2b[:, oc, bi, off:off + nsz],
                                            id16)
                        nc.vector.tensor_add(
                            out=ot[:nsz, oc * 128:(oc + 1) * 128],
                            in0=mpt[:nsz, :],
                            in1=x2sb[:nsz, bi, i, oc * 128:(oc + 1) * 128])
                    nc.scalar.dma_start(out=out_dram[b, off:off + nsz, :],
                                        in_=ot[:nsz])

        # ---------------- interleaved pipeline driver -----------------
        class Pump:
            def __init__(self, gen):
                self.g = gen
                self.alive = True
                self.tag = 99
                try:
                    self.tag = next(self.g)
                except StopIteration:
                    self.alive = False

            def step(self):
                try:
                    self.tag = next(self.g)
                except StopIteration:
                    self.alive = False
                    self.tag = 99

        def drain(pump):
            while pump.alive:
                pump.step()

        def co_run(gb, ga):
            cnt = 0
            while gb.alive or (ga is not None and ga.alive):
                if ga is None or not ga.alive:
                    gb.step()
                    continue
                if not gb.alive:
                    ga.step()
                    continue
                if gb.tag < ga.tag:
                    gb.step()
                elif ga.tag < gb.tag:
                    ga.step()
                else:
                    # same phase: 2 B chunks per A chunk
                    if cnt % 3 != 2:
                        gb.step()
                    else:
                        ga.step()
                    cnt += 1

        drain(Pump(stA(0)))
        for p in range(4):
            gb = Pump(stB(p))
            ga = Pump(stA(p + 1)) if p < 3 else None
            co_run(gb, ga)


def build_nc(h):
    from concourse import bacc
    nc = bacc.Bacc(None, target_bir_lowering=False, debug=False)
    d = {}

    def din(name, shape, dt):
        d[name] = nc.declare_dram_parameter(name, list(shape), dt, isOutput=False)

    din('x_shard', (BL, N, C), FP32)
    out_dram = nc.declare_dram_parameter('out', [BL, N, C], FP32, isOutput=True)

    din('WqT', (C, C), BF16); din('WkT', (C, C), BF16)
    din('WvT', (C, C), BF16); din('WoT', (C, C), BF16)
    din('W1q8', (6, 128, MMG, 3, 2, 128), F8)
    din('W1l8', (6, 128, MMG, 3, 2, 128), F8)
    din('W2q8', (NCH, 128, 12, 2, 128), F8)
    din('W2l8', (NCH, 128, 12, 2, 128), F8)
    din('bq', (128, NCH), FP32); din('bk', (128, NCH), FP32)
    din('bo', (128, NCH), FP32); din('b1', (128, MMCH), FP32)
    din('b2', (128, NCH), FP32)
    din('dww', (128, NCH, 9), FP32); din('dwb', (128, NCH), FP32)
    din('E8', (G, C), BF16); din('SelW', (2, 128, 13 * 128), BF16)
    din('sel2', (2, 128), BF16)
    din('onepm', (128, 4), BF16)
    din('refy', (N,), FP32); din('refx', (N,), FP32)
    din('rowi', (N,), FP32); din('colj', (N,), FP32)
    din('wyv', (C,), BF16); din('wxv', (C,), BF16)
    din('id16', (128, 128), BF16); din('id8', (128, 128), F8)
    if not h['ln1_trivial']:
        din('ln1_g', (C,), FP32); din('ln1_b', (C,), FP32)
    if not h['ln2_trivial']:
        din('ln2_g', (C,), FP32); din('ln2_b', (C,), FP32)
    if not h['offln_trivial']:
        din('offln_g', (C,), FP32); din('offln_b', (C,), FP32)
    if not h['bv_trivial']:
        din('bv', (128, C), FP32)

    with tile.TileContext(nc) as tc:
        emit(nc, tc, d, out_dram, h)
    nc.compile()
    return nc


_DECLARED = {'WqT', 'WkT', 'WvT', 'WoT', 'W1q8', 'W1l8', 'W2q8', 'W2l8',
             'bq', 'bk', 'bo', 'b1', 'b2', 'dww', 'dwb', 'E8', 'SelW',
             'sel2', 'onepm', 'refy', 'refx', 'wyv', 'wxv', 'id16', 'id8',
             'rowi', 'colj'}

_CACHE = {}


def kernel(**inputs):
    h = build_host_consts(inputs)
    if 'nc' not in _CACHE:
        _CACHE['nc'] = build_nc(h)
    nc = _CACHE['nc']

    declared = set(_DECLARED)
    for nm in ('ln1', 'ln2', 'offln'):
        if not h[nm + '_trivial']:
            declared |= {nm + '_g', nm + '_b'}
    if not h['bv_trivial']:
        declared.add('bv')
    shared = {k: v for k, v in h.items()
              if k in declared and isinstance(v, np.ndarray)}

    x = _f32(inputs['x'])
    in_maps = []
    for c in range(NCORES):
        m = dict(shared)
        m['x_shard'] = np.ascontiguousarray(x[c * BL:(c + 1) * BL])
        in_maps.append(m)
    res = run_bass_kernel_spmd(nc, in_maps, list(range(NCORES)))
    outs = [res.results[c]['out'] for c in range(NCORES)]
    return np.concatenate(outs, axis=0).astype(np.float32)
